# revision 29
# baseline (speedup 1.0000x reference)
"""Trainium2 Bass kernel for nn_BatchLinear (segmented path-indexed grouped linear, MoE-routed).

Math (per token b with expert e = w_id[b], 8 paths (i, j, k, alpha)):
    out[b, 128*k:+128] += alpha * x[b, 128*i:+128] @ W[e, seg j]  (each seg 128x128)

Strategy:
  - Host: route tokens by expert (the "all-to-all token dispatch"), pack each
    core's tokens feature-major ([expert, feature, token]) so the device
    matmuls need no transpose, prescale W segs 4-7 by the path coefficient 0.5.
  - Device (8 cores, data parallel, weights replicated): for each expert block
    and token tile, 8 fp32r matmuls accumulate the 4 output segments in PSUM
    (2 paths per output segment), copy to SBUF, DMA out.
  - Host: scatter rows back to original token order.
"""

import numpy as np

import concourse.bacc as bacc
import concourse.mybir as mybir
import concourse.tile as tile
from concourse.bass_utils import run_bass_kernel_spmd

N_CORES = 8
B = 32768
E = 4
U = V = 128
IN_STRIDE = 512
N_SEG = 4  # input/output feature segments
# out seg k <- (input seg, weight seg) x 2 contributions (coefficients folded
# into the prescaled weights: segs 4-7 are scaled by 0.5 on the host)
CONTRIB = {0: [(0, 0), (3, 7)], 1: [(1, 1), (0, 4)], 2: [(2, 2), (1, 5)], 3: [(3, 3), (2, 6)]}

F32 = mybir.dt.float32
F32R = mybir.dt.float32r
BF16 = mybir.dt.bfloat16

import ml_dtypes

# "f32"  : fp32 I/O, fp32r matmuls (absmax-rel err ~1.5e-4)
# "bf16in": bf16 x/w, fp32 y (err ~2.5e-3, ~30% less DMA)
# "bf16" : bf16 everything (err ~4e-3, ~half DMA)
MODE = __import__("os").environ.get("KERNEL_MODE", "f32")

_cache = {}


def _token_tiles(cap):
    """512-token tiles plus a 16-aligned remainder (fp32r needs 16-aligned
    free dims; the remainder shares its output DMA with the previous tile)."""
    assert cap % 16 == 0
    tiles = []
    t0 = 0
    while t0 < cap:
        T = min(512, cap - t0)
        tiles.append((t0, T))
        t0 += T
    return tiles


def _build(cap):
    """Build + schedule the per-core Bass program for per-(core,expert) capacity `cap`."""
    key = (cap, MODE)
    if key in _cache:
        return _cache[key]

    IN_DT = F32 if MODE in ("f32", "f32x") else BF16
    MM_DT = {"f32": F32R, "f32x": F32}.get(MODE, BF16)
    OUT_DT = BF16 if MODE == "bf16" else F32

    nc = bacc.Bacc("TRN2", target_bir_lowering=False, debug=False, num_devices=N_CORES)
    x = nc.dram_tensor("x", [E, IN_STRIDE, cap], IN_DT, kind="ExternalInput")
    # weights pre-packed on the host into the exact SBUF layout [u, (e j), v]
    w = nc.dram_tensor("w", [U, E * 8 * V], IN_DT, kind="ExternalInput")
    y = nc.dram_tensor("y", [E, IN_STRIDE, cap], OUT_DT, kind="ExternalOutput")

    # output/input slabs sized so every DMA run is >= ~2 KB:
    # f32: first 512 tokens alone (fast pipeline start) + rest;
    # bf16 (2-byte): one whole-expert slab (cap*2 bytes per run)
    if mybir.dt.size(IN_DT) == 4 and cap > 512:
        slabs = [(0, 512), (512, cap - 512)]
    else:
        slabs = [(0, cap)]

    def x_view(e, s0, S):
        return (
            x[e, :, s0 : s0 + S].rearrange("(s p) t -> p s t", p=128).bitcast(MM_DT)
        )

    def y_view(e, s0, S):
        return y[e, :, s0 : s0 + S].rearrange("(s p) t -> p s t", p=128)

    # keep all 4 expert blocks resident when SBUF allows (cap 1040 for the
    # reference routing); scale prefetch depth down for very skewed routings
    xbufs = 4 if cap <= 1536 else 2

    with tile.TileContext(nc) as tc:
        with (
            tc.tile_pool(name="wpool", bufs=1) as wp,
            tc.tile_pool(name="xin", bufs=xbufs) as xp,
            tc.tile_pool(name="yout", bufs=2) as yp,
            tc.tile_pool(name="ps", bufs=2, space="PSUM") as pp,
        ):
            wts = [wp.tile([U, 8, V], MM_DT, tag=f"w{e}", name=f"wt{e}") for e in range(E)]
            xs_slabs = []  # [e][slab] -> tile

            def load_w(e):
                nc.sync.dma_start(
                    wts[e][:],
                    w[:, e * 8 * V : (e + 1) * 8 * V]
                    .rearrange("u (j v) -> u j v", v=V)
                    .bitcast(MM_DT),
                )

            def load_x(e):
                tiles = []
                for si, (s0, S) in enumerate(slabs):
                    xt = xp.tile([128, N_SEG, S], MM_DT, tag=f"xs{si}")
                    nc.sync.dma_start(xt[:], x_view(e, s0, S))
                    tiles.append(xt)
                xs_slabs.append(tiles)

            # first-needed data first
            load_w(0)
            load_x(0)
            load_x(1)
            for e in range(1, E):
                load_w(e)
            load_x(2)
            load_x(3)

            # PE warm-up during the initial DMA wait: dummy matmuls flip the
            # HAM clock gate to 8/8 before the first real matmul arrives
            WU_DT = F32 if mybir.dt.size(IN_DT) == 4 else BF16
            n_warm = 6 if WU_DT == F32 else 20
            dwu = wp.tile([U, V], WU_DT, name="dwu")
            dxu = wp.tile([128, 512], WU_DT, name="dxu")
            nc.gpsimd.memset(dwu[:], 0.0)
            nc.gpsimd.memset(dxu[:], 0.0)
            ps_warm = pp.tile([128, N_SEG, 512], F32, tag="ps", name="ps_warm")
            for _ in range(n_warm):
                nc.tensor.matmul(ps_warm[:, 0, :], dwu[:], dxu[:], start=True, stop=True)

            ncopy = 0
            for e in range(E):
                for si, (s0, S) in enumerate(slabs):
                    ys = yp.tile([128, N_SEG, S], OUT_DT, tag=f"ys{si}")
                    for t0 in range(0, S, 512):
                        T = min(512, S - t0)
                        xt = xs_slabs[e][si]
                        ps = pp.tile([128, N_SEG, 512], F32, tag="ps")
                        for k in range(N_SEG):
                            (i1, j1), (i2, j2) = CONTRIB[k]
                            nc.tensor.matmul(
                                ps[:, k, :T],
                                wts[e][:, j1, :],
                                xt[:, i1, t0 : t0 + T],
                                start=True,
                                stop=False,
                            )
                            nc.tensor.matmul(
                                ps[:, k, :T],
                                wts[e][:, j2, :],
                                xt[:, i2, t0 : t0 + T],
                                start=False,
                                stop=True,
                            )
                        # drain all 4 banks in one strided copy; alternate engines
                        if ncopy % 2 == 0:
                            nc.vector.tensor_copy(ys[:, :, t0 : t0 + T], ps[:, :, :T])
                        else:
                            nc.scalar.copy(ys[:, :, t0 : t0 + T], ps[:, :, :T])
                        ncopy += 1
                    nc.sync.dma_start(y_view(e, s0, S), ys[:])

    nc.compile()
    _cache[key] = nc
    return nc


def _route(tensor_w_id):
    """Split each expert's tokens into N_CORES chunks. Returns (chunks, cap):
    chunks[c][e] = 1-D array of token indices for core c, expert e."""
    chunks = [[None] * E for _ in range(N_CORES)]
    max_n = 1
    for e in range(E):
        idx_e = np.flatnonzero(tensor_w_id == e)
        parts = np.array_split(idx_e, N_CORES)
        for c in range(N_CORES):
            chunks[c][e] = parts[c]
            max_n = max(max_n, len(parts[c]))
    cap = -(-max_n // 16) * 16
    return chunks, cap


def _run(tensor_in, tensor_w, tensor_w_id, trace=False):
    tensor_in = np.ascontiguousarray(tensor_in, dtype=np.float32)
    tensor_w = np.asarray(tensor_w, dtype=np.float32)
    tensor_w_id = np.asarray(tensor_w_id, dtype=np.int32)

    chunks, cap = _route(tensor_w_id)
    nc = _build(cap)

    # prescale: fold the 0.5 path coefficient into weight segs 4-7, and
    # pre-arrange into the SBUF layout [u, (e j), v] so the DMA is contiguous
    w_pack = tensor_w.reshape(E, 8, U, V).copy()
    w_pack[:, 4:] *= 0.5
    w_pack = np.ascontiguousarray(w_pack.transpose(2, 0, 1, 3)).reshape(U, E * 8 * V)

    # pack: gather + transpose to [E, feature, token] per core
    big_idx = np.zeros((N_CORES, E, cap), dtype=np.int64)
    for c in range(N_CORES):
        for e in range(E):
            idx = chunks[c][e]
            big_idx[c, e, : len(idx)] = idx
    xg = tensor_in[big_idx.reshape(-1)]  # [N_CORES*E*cap, 512]
    xg = xg.reshape(N_CORES, E, cap, IN_STRIDE).transpose(0, 1, 3, 2)  # -> [c, e, f, t]

    if MODE in ("f32", "f32x"):
        host_in_dt = np.float32
    else:
        host_in_dt = ml_dtypes.bfloat16
    w_pack = w_pack.astype(host_in_dt)
    in_maps = [
        {"x": np.ascontiguousarray(xg[c]).astype(host_in_dt), "w": w_pack}
        for c in range(N_CORES)
    ]
    import os

    kwargs = {}
    if trace:
        import shutil

        os.environ.pop("BASS_NEVER_TRACE", None)
        tmpdir = "/tmp/prof"
        shutil.rmtree(tmpdir, ignore_errors=True)
        os.makedirs(tmpdir, exist_ok=True)
        kwargs["tmpdir"] = tmpdir
    else:
        # a stray BASS_TRACE in the environment would route through the NTFF
        # profile hook, which this image lacks — force tracing off
        os.environ["BASS_NEVER_TRACE"] = "1"
    res = run_bass_kernel_spmd(nc, in_maps, list(range(N_CORES)), trace=trace, **kwargs)

    out = np.empty((B, IN_STRIDE), dtype=np.float32)
    for c in range(N_CORES):
        yc = np.asarray(res.results[c]["y"], dtype=np.float32)  # [E, 512, cap]
        for e in range(E):
            idx = chunks[c][e]
            if len(idx):
                out[idx] = yc[e, :, : len(idx)].T
    return out, res


def kernel(tensor_in, tensor_w, tensor_w_id):
    out, _ = _run(tensor_in, tensor_w, tensor_w_id)
    return out
